# revision 1
# baseline (speedup 1.0000x reference)
"""Bass/Trainium2 kernel for nn_BiSDA_37160057045272.

The reference module is a spiking (LIF) sparse-attention block.  Its final
stage is ``out = lif(attn_spike * v_spike)`` followed by a projection +
BatchNorm.  Both ``attn_spike`` and ``v_spike`` are Heaviside spikes in
{0, 1}, so the final LIF's input x is in [0, 1].  With the LIF update
``v <- v + (x - v)/tau`` (tau = 2, v0 = 0), the membrane potential after
T = 4 steps is bounded by 0.5 + 0.25 + 0.125 + 0.0625 = 0.9375 < V_TH = 1.0,
so the final LIF can NEVER fire, for ANY input values.  The last lif()
output is identically zero, the projection of zeros is zero, and
BatchNorm3d of a constant-zero tensor is ``0 * gamma + beta = beta``.

Hence the module computes, exactly, for every input:

    output[t, b, c, l, h, w] = p_beta[c]

(verified bit-exact against the jax reference for the spec inputs, for
random gammas/betas, and for 100x-scaled activations).

The kernel therefore broadcasts p_beta into the full output shape.  Each of
the 8 NeuronCores materializes 1/8 of the output (2 of the 16 T*B items,
i.e. a [2, 256, 8192] f32 shard = 16.8 MB) in device DRAM: p_beta is DMA'd
to SBUF, replicated across the free dimension on the vector engine, and
written out with large (multi-MB) DMAs that stripe across all 16 SDMA
engines.  The host concatenates the 8 shards into the full output.
"""

import numpy as np

import concourse.bacc as bacc
import concourse.mybir as mybir
import concourse.tile as tile
from concourse.bass_utils import run_bass_kernel_spmd

# Problem shapes (hardcoded per contract -- kernel.py must be self-contained).
T, B, C, Lt, Lh, Lw = 4, 4, 256, 8, 32, 32
N = Lt * Lh * Lw            # 8192 spatial positions
ITEMS = T * B               # 16 flattened (t, b) items
N_CORES = 8
IPC = ITEMS // N_CORES      # 2 items per core
P = 128                     # SBUF partitions
CT = C // P                 # 2 channel tiles
CHUNK = 4096                # free-dim elements per output DMA (2 MB each)

_CACHE: dict = {}
LAST_RESULTS = None         # BassKernelResults of the last run (for test harness)


def _build_nc():
    nc = bacc.Bacc("TRN2", target_bir_lowering=False, debug=False)
    p_beta = nc.dram_tensor("p_beta", (C,), mybir.dt.float32, kind="ExternalInput")
    out = nc.dram_tensor(
        "out", (IPC, C, N), mybir.dt.float32, kind="ExternalOutput"
    )
    out_ap = out.ap()
    with tile.TileContext(nc) as tc:
        with (
            tc.tile_pool(name="beta", bufs=1) as bpool,
            tc.tile_pool(name="big", bufs=CT) as gpool,
        ):
            # beta_sb[p, a] = p_beta[a*128 + p]
            beta_sb = bpool.tile([P, CT], mybir.dt.float32)
            nc.sync.dma_start(
                out=beta_sb[:, :],
                in_=p_beta.ap().rearrange("(a p) -> p a", p=P),
            )
            for ct in range(CT):
                big = gpool.tile([P, N], mybir.dt.float32)
                # Replicate the per-partition beta value across the free dim,
                # chunk by chunk so the first output DMA can start early.
                for j in range(0, N, CHUNK):
                    nc.vector.tensor_copy(
                        out=big[:, j : j + CHUNK],
                        in_=beta_sb[:, ct : ct + 1].to_broadcast([P, CHUNK]),
                    )
                for it in range(IPC):
                    for j in range(0, N, CHUNK):
                        nc.sync.dma_start(
                            out=out_ap[it, ct * P : (ct + 1) * P, j : j + CHUNK],
                            in_=big[:, j : j + CHUNK],
                        )
    nc.compile()
    return nc


def _get_nc():
    if "nc" not in _CACHE:
        _CACHE["nc"] = _build_nc()
    return _CACHE["nc"]


def kernel(**inputs) -> np.ndarray:
    global LAST_RESULTS
    p_beta = np.ascontiguousarray(np.asarray(inputs["p_beta"], dtype=np.float32))
    nc = _get_nc()
    in_maps = [{"p_beta": p_beta} for _ in range(N_CORES)]
    res = run_bass_kernel_spmd(nc, in_maps, core_ids=list(range(N_CORES)))
    LAST_RESULTS = res
    shards = [res.results[c]["out"] for c in range(N_CORES)]
    full = np.concatenate(shards, axis=0)          # [16, C, N]
    return full.reshape(T, B, C, Lt, Lh, Lw)


# revision 11
# speedup vs baseline: 1.2195x; 1.2195x over previous
"""Bass/Trainium2 kernel for nn_BiSDA_37160057045272.

The reference module is a spiking (LIF) sparse-attention block.  Its final
stage is ``out = lif(attn_spike * v_spike)`` followed by a projection +
BatchNorm.  Both ``attn_spike`` and ``v_spike`` are Heaviside spikes in
{0, 1}, so the final LIF's input x is in [0, 1].  With the LIF update
``v <- v + (x - v)/tau`` (tau = 2, v0 = 0), the membrane potential after
T = 4 steps is bounded by 0.5 + 0.25 + 0.125 + 0.0625 = 0.9375 < V_TH = 1.0,
so the final LIF can NEVER fire, for ANY input values.  The last lif()
output is identically zero, the projection of zeros is zero, and
BatchNorm3d of a constant-zero tensor is ``0 * gamma + beta = beta``.

Hence the module computes, exactly, for every input:

    output[t, b, c, l, h, w] = p_beta[c]

(verified bit-exact against the jax reference for the spec inputs, for
random gammas/betas, and for 100x-scaled activations).

The kernel therefore broadcasts p_beta into the full output shape.  Each of
the 8 NeuronCores materializes 1/8 of the output (2 of the 16 T*B items,
i.e. a [2, 256, 8192] f32 shard = 16.8 MB) in device DRAM: p_beta is DMA'd
to SBUF, replicated across the free dimension on the vector engine, and
written out with large (multi-MB) DMAs that stripe across all 16 SDMA
engines.  The host concatenates the 8 shards into the full output.
"""

import numpy as np

import concourse.bacc as bacc
import concourse.mybir as mybir
import concourse.tile as tile
from concourse.bass_utils import run_bass_kernel_spmd

# Problem shapes (hardcoded per contract -- kernel.py must be self-contained).
T, B, C, Lt, Lh, Lw = 4, 4, 256, 8, 32, 32
N = Lt * Lh * Lw            # 8192 spatial positions
ITEMS = T * B               # 16 flattened (t, b) items
N_CORES = 8
IPC = ITEMS // N_CORES      # 2 items per core
P = 128                     # SBUF partitions
CT = C // P                 # 2 channel tiles
FILL_CHUNK = 4096           # free-dim elements per SBUF fill instruction
DMA_CHUNK = 4096            # free-dim elements per output DMA (2 MB each)
FIRST_FILL = 1024           # small leading span so the first DMA starts early

_CACHE: dict = {}
LAST_RESULTS = None         # BassKernelResults of the last run (for test harness)


def _build_nc():
    nc = bacc.Bacc("TRN2", target_bir_lowering=False, debug=False)
    p_beta = nc.dram_tensor("p_beta", (C,), mybir.dt.float32, kind="ExternalInput")
    out = nc.dram_tensor(
        "out", (IPC, C, N), mybir.dt.float32, kind="ExternalOutput"
    )
    out_ap = out.ap()
    with tile.TileContext(nc) as tc:
        with (
            tc.tile_pool(name="beta", bufs=1) as bpool,
            tc.tile_pool(name="big", bufs=CT) as gpool,
        ):
            # beta_sb[p, a] = p_beta[a*128 + p]
            beta_sb = bpool.tile([P, CT], mybir.dt.float32)
            nc.sync.dma_start(
                out=beta_sb[:, :],
                in_=p_beta.ap().rearrange("(a p) -> p a", p=P),
            )

            def spans(first, rest):
                """[0:first], then `rest`-sized spans up to N."""
                out, j = [], 0
                if first and first < rest:
                    out.append((0, first))
                    j = first
                while j < N:
                    w = min(rest, N - j)
                    out.append((j, w))
                    j += w
                return out

            for ct in range(CT):
                big = gpool.tile([P, N], mybir.dt.float32)
                # Replicate the per-partition beta value across the free dim.
                # A small leading span lets the first output DMA start early.
                first = FIRST_FILL if ct == 0 else 0
                for j, w in spans(first, FILL_CHUNK):
                    nc.vector.tensor_copy(
                        out=big[:, j : j + w],
                        in_=beta_sb[:, ct : ct + 1].to_broadcast([P, w]),
                    )
                for it in range(IPC):
                    dma_first = FIRST_FILL if (ct == 0 and it == 0) else 0
                    for j, w in spans(dma_first, DMA_CHUNK):
                        nc.sync.dma_start(
                            out=out_ap[it, ct * P : (ct + 1) * P, j : j + w],
                            in_=big[:, j : j + w],
                        )
    nc.compile()
    return nc


def _get_nc():
    if "nc" not in _CACHE:
        _CACHE["nc"] = _build_nc()
    return _CACHE["nc"]


def kernel(**inputs) -> np.ndarray:
    global LAST_RESULTS
    p_beta = np.ascontiguousarray(np.asarray(inputs["p_beta"], dtype=np.float32))
    nc = _get_nc()
    in_maps = [{"p_beta": p_beta} for _ in range(N_CORES)]
    res = run_bass_kernel_spmd(nc, in_maps, core_ids=list(range(N_CORES)))
    LAST_RESULTS = res
    shards = [res.results[c]["out"] for c in range(N_CORES)]
    full = np.concatenate(shards, axis=0)          # [16, C, N]
    return full.reshape(T, B, C, Lt, Lh, Lw)
